# revision 13
# baseline (speedup 1.0000x reference)
"""Trainium2 Bass kernel for batched cross-attention.

Problem (hardcoded shapes):
  img_embeds:          (8, 4096, 512)  f32
  text_embeds:         (8, 512, 768)   f32
  text_attention_mask: (8, 512)        i32
  Wq (512,512), Wk (512,768), Wv (512,768), Wo (512,512), bo (512,)
  out:                 (8, 4096, 512)  f32

Sharding: data-parallel over batch B=8 -> one batch element per NeuronCore.
Weights replicated, pre-transposed on the host into the SBUF-friendly
[contract, free] layouts, and cached on device across calls (re-verified
with np.array_equal each call; changed weights trigger a re-upload, not a
recompile). Inputs are fed as zero-copy reshaped views of the full arrays;
the output placeholder operand is a persistent device buffer (the kernel
writes every output element, so its contents never matter).

Per-core algorithm (layouts chosen so the softmax denominator comes for
free and no transposes of big intermediates are needed):
  - PE-transpose text once: t^T; K^T = Wk^T-matmuls, V = t-matmuls
    (V stored per-head with an appended mask-column).
  - per 512-query block: PE-transpose x chunk, Q^T = Wq^T @ x^T.
  - per head: scores^T[j,i] = K_h^T.T @ Q_h^T (K=64), then
    exp(scale*s) on ACT, masked via a multiplicative 0/1 factor folded
    into V_ext, then attended^T[vd,i] = V_ext.T @ exp accumulated over j.
    Rows [HD:2HD] of attended^T are the softmax denominator.
    reciprocal + normalize on DVE.
  - Y[i,od] = attn^T.T @ Wo^T (+ bo via a K=1 accumulation matmul).

Matmuls run as float32r (full fp32 data; 1 cycle/row on TRN2 when the
moving free dim >= 256).
"""

import os
from contextlib import ExitStack

import numpy as np

import concourse.bass as bass
import concourse.tile as tile
from concourse import bacc, mybir
from concourse.masks import make_identity

F32 = mybir.dt.float32
F32R = mybir.dt.float32r
I32 = mybir.dt.int32

B, N_IMG, N_TXT = 8, 4096, 512
IMG_DIM, TEXT_DIM, H, HD = 512, 768, 8, 64
SCALE = float((TEXT_DIM // H) ** -0.5)
P = 128
N_CORES = 8

IB = N_IMG // 512  # 8 query blocks of 512
NJC = N_TXT // P   # 4 key chunks of 128


def _r(ap):
    """fp32 -> float32r view for full-rate PE matmuls."""
    return ap.bitcast(F32R)


def _build_nc(repeat: int = 1) -> bass.Bass:
    nc = bacc.Bacc("TRN2", target_bir_lowering=False, debug=False)

    img = nc.dram_tensor("img", [N_IMG, IMG_DIM], F32, kind="ExternalInput").ap()
    txt = nc.dram_tensor("txt", [N_TXT, TEXT_DIM], F32, kind="ExternalInput").ap()
    msk = nc.dram_tensor("msk", [N_TXT], F32, kind="ExternalInput").ap()
    wqt = nc.dram_tensor("wqt", [P, 4, 512], F32, kind="ExternalInput").ap()
    wkt = nc.dram_tensor("wkt", [P, 6, 512], F32, kind="ExternalInput").ap()
    wvt = nc.dram_tensor("wvt", [P, 6, 512], F32, kind="ExternalInput").ap()
    wot = nc.dram_tensor("wot", [P, 4, 512], F32, kind="ExternalInput").ap()
    bo = nc.dram_tensor("bo", [1, 512], F32, kind="ExternalInput").ap()
    out = nc.dram_tensor("out", [N_IMG, IMG_DIM], F32, kind="ExternalOutput").ap()

    with tile.TileContext(nc) as tc:
        with ExitStack() as ctx:
            _body(ctx, tc, img, txt, msk, wqt, wkt, wvt, wot, bo, out, repeat)
    nc.compile()
    return nc


def _body(ctx, tc, img, txt, msk, wqt, wkt, wvt, wot, bo, out, repeat=1):
    nc = tc.nc
    njc = NJC
    Exp = mybir.ActivationFunctionType.Exp

    img_r = img.rearrange("(n p) d -> p n d", p=P)  # n = 32 row-chunks
    out_r = out.rearrange("(n p) d -> p n d", p=P)

    const = ctx.enter_context(tc.tile_pool(name="const", bufs=1))
    ps = ctx.enter_context(tc.tile_pool(name="ps", bufs=8, space="PSUM"))

    identity = const.tile([P, P], F32, tag="identity")
    make_identity(nc, identity)

    # ---- weights arrive pre-transposed; DMA into F32 staging, then a
    # single on-chip copy into F32R tiles (fp32r matmul operands must be
    # written by an fp32r-rounding instruction per the BIR verifier).
    WqT = const.tile([P, 4, 512], F32R, tag="WqT")  # [d, qd]
    WoT = const.tile([P, 4, 512], F32R, tag="WoT")  # [c, od]
    WkT = const.tile([P, 6, 512], F32R, tag="WkT")  # [td, kd]
    WvT = const.tile([P, 6, 512], F32R, tag="WvT")  # [td, vd]

    tT = const.tile([P, 6, N_TXT], F32R, tag="tT")    # [td, j]
    KT = const.tile([P, 4, N_TXT], F32R, tag="KT")    # [kd, j]
    Vx = const.tile([P, njc, H, 2 * HD], F32R, tag="Vx")  # [j%, jc, h, vd|mask]
    bo_sb = const.tile([1, 512], F32, tag="bo_sb")
    bo_r = const.tile([1, 512], F32R, tag="bo_r")
    ones = const.tile([1, P], F32R, tag="ones")
    ones_f = const.tile([P, HD], F32, tag="ones_f")
    nc.any.memset(ones_f, 1.0)
    # mask as multiplicative factor on V_ext rows, laid out [p, jc].
    maskb_f = const.tile([P, njc], F32, tag="mf")
    mask_row = const.tile([njc, P], F32, tag="mrow")
    nc.sync.dma_start(mask_row, msk.rearrange("(c p) -> c p", p=P))
    mps = ps.tile([P, njc], F32, tag="ps", bufs=2, name="mps")
    nc.tensor.transpose(mps, mask_row, identity[:njc, :njc])
    nc.vector.tensor_copy(maskb_f, mps)
    nc.vector.tensor_copy(ones, ones_f[0:1, 0:1].broadcast_to([1, P]))
    for jc in range(njc):
        nc.vector.tensor_scalar_mul(
            Vx[:, jc, :, HD:],
            ones_f[:, None, :].broadcast_to([P, H, HD]),
            maskb_f[:, jc : jc + 1],
        )
    nc.gpsimd.dma_start(bo_sb, bo)
    nc.vector.tensor_copy(bo_r, bo_sb)

    def transpose_in(dst, src_chunks, n_out_chunks, n_in_chunks, evict_engine):
        """dst[p, oc, ic*128+q] = src[q, ic, oc*128+p]."""
        for oc in range(n_out_chunks):
            pst = ps.tile([P, 512], F32, tag="at", bufs=2, name=f"pst_{oc}")
            for ic in range(n_in_chunks):
                nc.tensor.transpose(
                    pst[:, ic * P : (ic + 1) * P],
                    src_chunks[:, ic, oc * P : (oc + 1) * P],
                    identity,
                )
            evict_engine.tensor_copy(dst[:, oc, : n_in_chunks * P], pst[:, : n_in_chunks * P])

    # ---- one-time setup: weight staging copies, text transpose, K^T, V
    wload = ctx.enter_context(tc.tile_pool(name="wload", bufs=2))
    for dram_src, dst, nch in (
        (wqt, WqT, 4),
        (wkt, WkT, 6),
        (wvt, WvT, 6),
        (wot, WoT, 4),
    ):
        stg = wload.tile([P, nch, 512], F32, tag="wl")
        nc.sync.dma_start(stg, dram_src)
        nc.vector.tensor_copy(dst, stg)

    t_sb = wload.tile([P, njc, 768], F32, tag="wl")
    nc.sync.dma_start(t_sb, txt.rearrange("(c p) d -> p c d", p=P))
    transpose_in(tT, t_sb, 6, njc, nc.vector)

    # K^T[kd, j] = sum_td WkT[td, kd] * tT[td, j]
    for kc in range(4):
        pkt = ps.tile([P, 512], F32, tag="ps", bufs=2, name=f"pkt_{kc}")
        for t6 in range(6):
            nc.tensor.matmul(
                pkt[:, :N_TXT],
                WkT[:, t6, kc * P : (kc + 1) * P],
                tT[:, t6, :],
                start=(t6 == 0),
                stop=(t6 == 5),
            )
        nc.vector.tensor_copy(KT[:, kc, :], pkt[:, :N_TXT])

    # V[j, vd] = sum_td tT[td, j] * WvT[td, vd]; per-head columns, mask applied
    for jc in range(njc):
        pv = ps.tile([P, 512], F32, tag="ps", bufs=2, name=f"pv_{jc}")
        for t6 in range(6):
            nc.tensor.matmul(
                pv,
                tT[:, t6, jc * P : (jc + 1) * P],
                WvT[:, t6, :],
                start=(t6 == 0),
                stop=(t6 == 5),
            )
        nc.vector.tensor_scalar_mul(
            Vx[:, jc, :, :HD],
            pv.rearrange("p (h v) -> p h v", h=H),
            maskb_f[:, jc : jc + 1],
        )

    # ---- pipelined pools for the main loop
    xload = ctx.enter_context(tc.tile_pool(name="xload", bufs=2))
    xtp = ctx.enter_context(tc.tile_pool(name="xtp", bufs=2))
    qtp = ctx.enter_context(tc.tile_pool(name="qtp", bufs=2))
    exp = ctx.enter_context(tc.tile_pool(name="exw", bufs=3))
    anp = ctx.enter_context(tc.tile_pool(name="anp", bufs=2))
    asp = ctx.enter_context(tc.tile_pool(name="asp", bufs=3))
    ysp = ctx.enter_context(tc.tile_pool(name="ysp", bufs=3))

    def _main_loop():
      for ib in range(IB):
        x_sb = xload.tile([P, 4, 512], F32, tag="x")
        nc.sync.dma_start(x_sb, img_r[:, ib * 4 : (ib + 1) * 4, :])

        # x^T for this 512-query block
        xT = xtp.tile([P, 4, 512], F32R, tag="xT")  # [d, i]
        transpose_in(xT, x_sb, 4, 4, nc.vector)

        # Q^T[qd, i] = sum_d WqT[d, qd] * xT[d, i]
        qt = qtp.tile([P, 4, 512], F32R, tag="qt")  # [qd, i]
        for qc in range(4):
            pq = ps.tile([P, 512], F32, tag="ps", bufs=2, name=f"pq_{qc}")
            for dc in range(4):
                nc.tensor.matmul(
                    pq,
                    WqT[:, dc, qc * P : (qc + 1) * P],
                    xT[:, dc, :],
                    start=(dc == 0),
                    stop=(dc == 3),
                )
            nc.vector.tensor_copy(qt[:, qc, :], pq)

        attn = anp.tile([P, 4, 512], F32R, tag="attn")  # [c, i] normalized att^T

        def head_scores(h):
            po = (h % 2) * HD
            hc = h // 2
            qh = qt[po : po + HD, hc, :]  # [64, 512]
            ex = exp.tile([P, njc * 512], F32R, tag="ex", name="ex")
            for jh in range(njc // 2):
                sc = ps.tile([P, 1024], F32, tag="sc", bufs=2, name="sc")
                for k in range(2):
                    jc = jh * 2 + k
                    nc.tensor.matmul(
                        sc[:, k * 512 : (k + 1) * 512],
                        KT[po : po + HD, hc, jc * P : (jc + 1) * P],
                        qh,
                    )
                nc.scalar.activation(
                    ex[:, jh * 1024 : (jh + 1) * 1024], sc, Exp, scale=SCALE
                )
            return ex

        def head_attend(h, ex):
            po = (h % 2) * HD
            hc = h // 2
            at = ps.tile([P, 512], F32, tag="at", bufs=2, name="at")
            for jc in range(njc):
                nc.tensor.matmul(
                    at,
                    Vx[:, jc, h, :],
                    ex[:, jc * 512 : (jc + 1) * 512],
                    start=(jc == 0),
                    stop=(jc == njc - 1),
                )
            # rows [HD:2*HD] of `at` are the softmax denominator, replicated
            rec = asp.tile([HD, 512], F32, tag="rec")
            nc.vector.reciprocal(rec, at[HD:, :])
            nc.vector.tensor_mul(attn[po : po + HD, hc, :], at[:HD, :], rec)

        # software pipeline: head h's scores/exp overlap head h-1's attend
        prev = None
        for h in range(H):
            ex = head_scores(h)
            if prev is not None:
                head_attend(prev[0], prev[1])
            prev = (h, ex)
        head_attend(prev[0], prev[1])

        # Y[i, od] = sum_c attn[c, i] * WoT[c, od] + bo
        for mc in range(4):
            py = ps.tile([P, 512], F32, tag="ps", bufs=2, name=f"py_{mc}")
            for cc in range(4):
                nc.tensor.matmul(
                    py,
                    attn[:, cc, mc * P : (mc + 1) * P],
                    WoT[:, cc, :],
                    start=(cc == 0),
                    stop=False,
                )
            nc.tensor.matmul(py, ones, bo_r, start=False, stop=True)
            y_sb = ysp.tile([P, 512], F32, tag="y")
            nc.vector.tensor_copy(y_sb, py)
            nc.sync.dma_start(out_r[:, ib * 4 + mc, :], y_sb)

    if repeat == 1:
        _main_loop()
    else:
        with tc.For_i(0, repeat, 1):
            _main_loop()


# ---------------------------------------------------------------------------
# Host-side runner: minimal per-call overhead.
#   - jit (shard_map over 8 cores) cached per `repeat`
#   - weights pre-transposed + device-cached (np.array_equal re-check per call)
#   - inputs passed as zero-copy views; output slot is a persistent dev buffer
# ---------------------------------------------------------------------------

_RUNNERS = {}
_WCACHE = {}


def _get_runner(repeat: int = 1):
    key = repeat
    if key in _RUNNERS:
        return _RUNNERS[key]

    import jax
    from jax.sharding import Mesh, PartitionSpec
    from jax.experimental.shard_map import shard_map
    from concourse import bass2jax

    nc = _build_nc(repeat=repeat)
    bass2jax.install_neuronx_cc_hook()

    partition_name = nc.partition_id_tensor.name if nc.partition_id_tensor else None
    in_names = []
    out_names = []
    out_avals = []
    zero_out_shapes = []
    for alloc in nc.m.functions[0].allocations:
        if not isinstance(alloc, mybir.MemoryLocationSet):
            continue
        name = alloc.memorylocations[0].name
        if alloc.kind == "ExternalInput":
            if name != partition_name:
                in_names.append(name)
        elif alloc.kind == "ExternalOutput":
            shape = tuple(alloc.tensor_shape)
            dtype = mybir.dt.np(alloc.dtype)
            out_names.append(name)
            out_avals.append(jax.core.ShapedArray(shape, dtype))
            zero_out_shapes.append((shape, dtype))
    n_params = len(in_names)
    n_outs = len(out_names)
    all_names = list(in_names) + list(out_names)
    if partition_name is not None:
        all_names.append(partition_name)

    def _bodyfn(*args):
        operands = list(args)
        if partition_name is not None:
            operands.append(bass2jax.partition_id_tensor())
        outs = bass2jax._bass_exec_p.bind(
            *operands,
            out_avals=tuple(out_avals),
            in_names=tuple(all_names),
            out_names=tuple(out_names),
            lowering_input_output_aliases=(),
            sim_require_finite=True,
            sim_require_nnan=True,
            nc=nc,
        )
        return tuple(outs)

    devices = jax.devices()[:N_CORES]
    mesh = Mesh(np.asarray(devices), ("core",))
    sharded = jax.jit(
        shard_map(
            _bodyfn,
            mesh=mesh,
            in_specs=(PartitionSpec("core"),) * (n_params + n_outs),
            out_specs=(PartitionSpec("core"),) * n_outs,
            check_rep=False,
        ),
        keep_unused=True,
    )

    # persistent output-slot placeholder: the kernel writes every element of
    # `out`, so the contents of this operand are never observable.
    from jax.sharding import NamedSharding

    sh = NamedSharding(mesh, PartitionSpec("core"))
    dummies = [
        jax.device_put(
            np.zeros((N_CORES * s[0],) + tuple(s[1:]), dt), sh
        )
        for (s, dt) in zero_out_shapes
    ]
    jax.block_until_ready(dummies)

    _RUNNERS[key] = (sharded, in_names, out_names, zero_out_shapes, nc, dummies, sh)
    return _RUNNERS[key]


def _transpose_weights(Wq, Wk, Wv, Wo, bo):
    """Host-side pre-transpose into the [p, chunk, free] SBUF layouts."""
    def to_pcf(wT, nchunk):
        # wT: [contract, free] -> [p, chunk, free] with contract = chunk*128+p
        return np.ascontiguousarray(
            wT.reshape(nchunk, P, wT.shape[1]).transpose(1, 0, 2)
        )

    wqt = to_pcf(Wq.T, 4)   # [d, qd]
    wkt = to_pcf(Wk.T, 6)   # [td, kd]
    wvt = to_pcf(Wv.T, 6)   # [td, vd]
    wot = to_pcf(Wo.T, 4)   # [c, od]
    bo2 = np.ascontiguousarray(bo.reshape(1, 512))
    return wqt, wkt, wvt, wot, bo2


def _ensure_weights(Wq, Wk, Wv, Wo, bo, sh):
    """Return device-resident replicated weight arrays, re-uploading only
    when the values change."""
    import jax

    global _WCACHE
    c = _WCACHE
    if c and all(
        np.array_equal(c["host"][i], w) for i, w in enumerate((Wq, Wk, Wv, Wo, bo))
    ):
        return c["dev"]

    host = tuple(np.asarray(w, dtype=np.float32) for w in (Wq, Wk, Wv, Wo, bo))
    wqt, wkt, wvt, wot, bo2 = _transpose_weights(*host)
    dev = []
    for arr in (wqt, wkt, wvt, wot, bo2):
        rep = np.ascontiguousarray(
            np.broadcast_to(arr[None], (N_CORES,) + arr.shape)
        ).reshape((N_CORES * arr.shape[0],) + arr.shape[1:])
        dev.append(jax.device_put(rep, sh))
    jax.block_until_ready(dev)
    _WCACHE = {"host": host, "dev": dev}
    return dev


def kernel(img_embeds, text_embeds, text_attention_mask, Wq, Wk, Wv, Wo, bo):
    import jax

    sharded, in_names, out_names, zero_out_shapes, nc, dummies, sh = _get_runner(1)
    w_dev = _ensure_weights(Wq, Wk, Wv, Wo, bo, sh)

    img = np.ascontiguousarray(np.asarray(img_embeds, dtype=np.float32)).reshape(
        B * N_IMG, IMG_DIM
    )
    txt = np.ascontiguousarray(np.asarray(text_embeds, dtype=np.float32)).reshape(
        B * N_TXT, TEXT_DIM
    )
    mskf = np.asarray(text_attention_mask).astype(np.float32).reshape(B * N_TXT)

    outs = sharded(img, txt, mskf, *w_dev, *dummies)
    out = np.asarray(outs[0]).reshape(B, N_IMG, IMG_DIM)
    return out


# ---------------------------------------------------------------------------
# Benchmark helpers (used by test.py)
# ---------------------------------------------------------------------------


def _dev_inputs(inputs, repeat: int = 1):
    """Device-resident input list for the given runner."""
    import jax

    sharded, in_names, out_names, zero_out_shapes, nc, dummies, sh = _get_runner(repeat)
    w_dev = _ensure_weights(
        inputs["Wq"], inputs["Wk"], inputs["Wv"], inputs["Wo"], inputs["bo"], sh
    )
    img = np.ascontiguousarray(
        np.asarray(inputs["img_embeds"], dtype=np.float32)
    ).reshape(B * N_IMG, IMG_DIM)
    txt = np.ascontiguousarray(
        np.asarray(inputs["text_embeds"], dtype=np.float32)
    ).reshape(B * N_TXT, TEXT_DIM)
    mskf = np.asarray(inputs["text_attention_mask"]).astype(np.float32).reshape(
        B * N_TXT
    )
    dev = [jax.device_put(a, sh) for a in (img, txt, mskf)]
    jax.block_until_ready(dev)
    return sharded, dev + list(w_dev) + list(dummies)


def bench_repeat(inputs, repeat: int = 25, iters: int = 12):
    """Device-time via an in-NEFF For_i repeat loop: (t[repeat] - t[1]) /
    (repeat - 1)."""
    import time
    import jax

    runs = {}
    for rep in (1, repeat):
        sharded, args = _dev_inputs(inputs, rep)
        o = sharded(*args)
        jax.block_until_ready(o)
        runs[rep] = (sharded, args)

    times = {1: [], repeat: []}
    for _ in range(iters):
        for rep in (1, repeat):
            sharded, args = runs[rep]
            t0 = time.perf_counter()
            o = sharded(*args)
            jax.block_until_ready(o)
            times[rep].append(time.perf_counter() - t0)
    per = (min(times[repeat]) - min(times[1])) / (repeat - 1)
    return per, times
